# revision 6
# baseline (speedup 1.0000x reference)
"""KoLeo loss kernel for Trainium2 (8 NeuronCores, Bass/Tile).

loss = mean_i( -log( min_{j != i} ||x_i - x_j|| + eps ) ) for x: (8192, 512) f32.

Data-parallel over rows, 1024 rows/core. Per core, for each of 8x16 (m, n)
tiles of the local Gram block:
  variant "act_init":
    PSUM tile is initialized by a ScalarE copy with w0 = -0.5*sq_j (diag tile:
    -BIG at the diagonal), then 4 bf16 matmuls accumulate +g on top
    (start=False rides the has_written bits, which engine writes preserve;
    8 warm-up matmuls pin them set). Then one VectorE row-max per tile:
      max_j (g - 0.5 sq_j)  ->  d2_i = sq_i - 2*max  == min_{j!=i} d2_ij.
  variant "mm5":
    4 bf16 matmuls of -2g (pre-scaled lhsT) + a 5th K=2 matmul adding
    ones x [sq_hi; sq_lo], then a VectorE row-min; the diagonal is masked by
    adding +BIG to the 128-wide diag band in separate small ops.

Each core works on a column-rotated copy of X^T (rotated by 1024*core) so the
diagonal lands at rotated columns [128m, 128m+128) on every core -> identical
SPMD program; only input data differs. Device outputs ln(d2min) per local
row; host does -0.5 * mean.
"""

import os
from contextlib import ExitStack

import numpy as np

os.environ.setdefault("JAX_PLATFORMS", "axon,cpu")

import ml_dtypes  # noqa: E402

import concourse.bass as bass  # noqa: E402
import concourse.tile as tile  # noqa: E402
from concourse import mybir  # noqa: E402
from concourse.bass_utils import run_bass_kernel_spmd  # noqa: E402

B, D = 8192, 512
N_CORES = 8
RPC = B // N_CORES            # rows per core: 1024
M_TILES = RPC // 128          # 8
N_TILES = B // 512            # 16
K_TILES = D // 128            # 4
BIG = 1.0e30
F32 = mybir.dt.float32
BF16 = mybir.dt.bfloat16

VARIANT = os.environ.get("KOLEO_VARIANT", "act_init")


def _build_act_init() -> bass.Bass:
    nc = bass.Bass("TRN2", target_bir_lowering=False, debug=False,
                   num_devices=N_CORES)
    xt = nc.dram_tensor("xt", [K_TILES, 128, B], BF16,
                        kind="ExternalInput").ap()
    sqh = nc.dram_tensor("sqh", [128, B], F32, kind="ExternalInput").ap()
    sqhd = nc.dram_tensor("sqhd", [128, M_TILES * 512], F32,
                          kind="ExternalInput").ap()
    sqloc = nc.dram_tensor("sqloc", [128, M_TILES], F32,
                           kind="ExternalInput").ap()
    out = nc.dram_tensor("lnd2", [128, M_TILES], F32,
                         kind="ExternalOutput").ap()

    with tile.TileContext(nc) as tc, ExitStack() as ctx:
        big = ctx.enter_context(tc.tile_pool(name="big", bufs=1))
        small = ctx.enter_context(tc.tile_pool(name="small", bufs=1))
        pp = ctx.enter_context(tc.tile_pool(name="pp", bufs=8, space="PSUM"))

        xt_sb = [big.tile([128, B], BF16, tag=f"xt_{k}", bufs=1,
                          name=f"xt_{k}") for k in range(K_TILES)]
        sqh_sb = big.tile([128, B], F32, tag="sqh", bufs=1)
        sqhd_sb = big.tile([128, M_TILES * 512], F32, tag="sqhd", bufs=1)
        sqloc_sb = small.tile([128, M_TILES], F32, tag="sqloc", bufs=1)
        maxb = small.tile([128, M_TILES * N_TILES], F32, tag="maxb", bufs=1)
        d2a = small.tile([128, M_TILES], F32, tag="d2a", bufs=1)
        d2b = small.tile([128, M_TILES], F32, tag="d2b", bufs=1)
        lnout = small.tile([128, M_TILES], F32, tag="lnout", bufs=1)
        scr_a = small.tile([1, 1], F32, tag="scr_a", bufs=1)
        scr_b = small.tile([1, 1], F32, tag="scr_b", bufs=1)
        scr_c = small.tile([1, 1], F32, tag="scr_c", bufs=1)
        scr2 = small.tile([1, 1], F32, tag="scr2", bufs=1)

        for k in range(K_TILES):
            nc.sync.dma_start(xt_sb[k][:], xt[k])
        nc.sync.dma_start(sqh_sb[:], sqh[:])
        nc.sync.dma_start(sqhd_sb[:], sqhd[:])
        nc.sync.dma_start(sqloc_sb[:], sqloc[:])

        # Wait-absorbers: dedicated first-touches so hot-loop instructions
        # need at most one attached sync wait (walrus AC/DVE struct limit).
        nc.scalar.activation(scr_a[:], sqh_sb[0:1, 0:1],
                             mybir.ActivationFunctionType.Copy)
        nc.scalar.activation(scr_b[:], sqhd_sb[0:1, 0:1],
                             mybir.ActivationFunctionType.Copy)
        nc.scalar.activation(scr_c[:], scr_a[:],
                             mybir.ActivationFunctionType.Ln)  # const-bias load
        nc.vector.tensor_copy(scr2[:], sqloc_sb[0:1, 0:1])

        # Warm-up matmuls: pin has_written on every element of all 8 banks so
        # the start=False groups below accumulate onto ScalarE-written init.
        for b_ in range(8):
            ps = pp.tile([128, 512], F32, tag="ps", bufs=8, name=f"warm_{b_}")
            nc.tensor.matmul(ps[:], xt_sb[0][:, 0:128], xt_sb[0][:, 0:512],
                             start=True, stop=True)

        for n in range(N_TILES):
            for m in range(M_TILES):
                ps = pp.tile([128, 512], F32, tag="ps", bufs=8,
                             name=f"ps_{n}_{m}")
                nd = m // 4               # n-tile holding the diagonal
                init = (sqhd_sb[:, m * 512:(m + 1) * 512] if n == nd
                        else sqh_sb[:, n * 512:(n + 1) * 512])
                nc.scalar.activation(ps[:], init,
                                     mybir.ActivationFunctionType.Copy)
                for k in range(K_TILES):
                    nc.tensor.matmul(
                        ps[:],
                        xt_sb[k][:, m * 128:(m + 1) * 128],  # lhsT: local cols
                        xt_sb[k][:, n * 512:(n + 1) * 512],
                        start=False,
                        stop=(k == K_TILES - 1),
                        skip_group_check=True,
                    )
                col = m * N_TILES + n
                nc.vector.tensor_reduce(
                    out=maxb[:, col:col + 1], in_=ps[:],
                    axis=mybir.AxisListType.X, op=mybir.AluOpType.max)

        for m in range(M_TILES):
            nc.vector.tensor_reduce(
                out=d2a[:, m:m + 1],
                in_=maxb[:, m * N_TILES:(m + 1) * N_TILES],
                axis=mybir.AxisListType.X, op=mybir.AluOpType.max)
        nc.vector.tensor_scalar_mul(d2b[:], d2a[:], -2.0)
        nc.vector.tensor_add(d2a[:], d2b[:], sqloc_sb[:])      # + sq_i
        nc.vector.tensor_scalar_max(d2b[:], d2a[:], 1e-12)     # NaN guard
        nc.scalar.activation(lnout[:], d2b[:],
                             mybir.ActivationFunctionType.Ln)
        nc.sync.dma_start(out[:], lnout[:])
    return nc


def _build_mm5() -> bass.Bass:
    nc = bass.Bass("TRN2", target_bir_lowering=False, debug=False,
                   num_devices=N_CORES)
    xt = nc.dram_tensor("xt", [K_TILES, N_TILES, 128, 512], BF16,
                        kind="ExternalInput").ap()
    xlt2 = nc.dram_tensor("xlt2", [K_TILES, 128, RPC], BF16,
                          kind="ExternalInput").ap()
    sqr = nc.dram_tensor("sqr", [2, B], BF16, kind="ExternalInput").ap()
    ones2 = nc.dram_tensor("ones2", [2, 128], BF16, kind="ExternalInput").ap()
    diagc = nc.dram_tensor("diagc", [128, 128], F32, kind="ExternalInput").ap()
    sqloc = nc.dram_tensor("sqloc", [128, M_TILES], F32,
                           kind="ExternalInput").ap()
    out = nc.dram_tensor("lnd2", [128, M_TILES], F32,
                         kind="ExternalOutput").ap()

    NCOL = N_TILES + 2  # per-m minb cols: 15 regular + up to 3 diag pieces
    with tile.TileContext(nc) as tc, ExitStack() as ctx:
        xt_pool = ctx.enter_context(tc.tile_pool(name="xtp", bufs=K_TILES * N_TILES))
        aux = ctx.enter_context(tc.tile_pool(name="aux", bufs=1))
        band_pool = ctx.enter_context(tc.tile_pool(name="bandp", bufs=2))
        pp = ctx.enter_context(tc.tile_pool(name="pp", bufs=8, space="PSUM"))

        xt_sb = [[xt_pool.tile([128, 512], BF16, tag="xt",
                               bufs=K_TILES * N_TILES, name=f"xt_{k}_{n}")
                  for n in range(N_TILES)] for k in range(K_TILES)]
        xlt2_sb = [aux.tile([128, RPC], BF16, tag=f"xlt2_{k}", bufs=1,
                            name=f"xlt2_{k}") for k in range(K_TILES)]
        sqr_sb = aux.tile([2, B], BF16, tag="sqr", bufs=1)
        ones2_sb = aux.tile([2, 128], BF16, tag="ones2", bufs=1)
        diagc_sb = aux.tile([128, 128], F32, tag="diagc", bufs=1)
        sqloc_sb = aux.tile([128, M_TILES], F32, tag="sqloc", bufs=1)
        minb = aux.tile([128, M_TILES * NCOL], F32, tag="minb", bufs=1)
        d2a = aux.tile([128, M_TILES], F32, tag="d2a", bufs=1)
        d2b = aux.tile([128, M_TILES], F32, tag="d2b", bufs=1)
        lnout = aux.tile([128, M_TILES], F32, tag="lnout", bufs=1)

        nc.sync.dma_start(sqloc_sb[:], sqloc[:])
        nc.sync.dma_start(sqr_sb[:], sqr[:])
        nc.sync.dma_start(ones2_sb[:], ones2[:])
        nc.sync.dma_start(diagc_sb[:], diagc[:])
        for k in range(K_TILES):
            nc.sync.dma_start(xlt2_sb[k][:], xlt2[k])
        for n in range(N_TILES):
            for k in range(K_TILES):
                nc.sync.dma_start(xt_sb[k][n][:], xt[k, n])

        nc.vector.memset(minb[:], BIG)

        for n in range(N_TILES):
            for m in range(M_TILES):
                ps = pp.tile([128, 512], F32, tag="ps", bufs=8,
                             name=f"ps_{n}_{m}")
                for k in range(K_TILES):
                    nc.tensor.matmul(
                        ps[:],
                        xlt2_sb[k][:, m * 128:(m + 1) * 128],
                        xt_sb[k][n][:],
                        start=(k == 0),
                        stop=False,
                    )
                nc.tensor.matmul(                      # += ones (x) sq rows
                    ps[:],
                    ones2_sb[:],
                    sqr_sb[:, n * 512:(n + 1) * 512],
                    start=False,
                    stop=True,
                )
                nd = m // 4
                o = 128 * (m % 4)
                cb = m * NCOL
                if n != nd:
                    col = cb + n - (1 if n > nd else 0)
                    nc.vector.tensor_reduce(
                        out=minb[:, col:col + 1], in_=ps[:],
                        axis=mybir.AxisListType.X, op=mybir.AluOpType.min)
                else:
                    band = band_pool.tile([128, 128], F32, tag="band", bufs=2,
                                          name=f"band_{m}")
                    nc.vector.tensor_add(band[:], ps[:, o:o + 128], diagc_sb[:])
                    nc.vector.tensor_reduce(
                        out=minb[:, cb + 15:cb + 16], in_=band[:],
                        axis=mybir.AxisListType.X, op=mybir.AluOpType.min)
                    if o > 0:
                        nc.vector.tensor_reduce(
                            out=minb[:, cb + 16:cb + 17], in_=ps[:, 0:o],
                            axis=mybir.AxisListType.X, op=mybir.AluOpType.min)
                    if o < 384:
                        nc.vector.tensor_reduce(
                            out=minb[:, cb + 17:cb + 18],
                            in_=ps[:, o + 128:512],
                            axis=mybir.AxisListType.X, op=mybir.AluOpType.min)

        for m in range(M_TILES):
            nc.vector.tensor_reduce(
                out=d2a[:, m:m + 1],
                in_=minb[:, m * NCOL:(m + 1) * NCOL],
                axis=mybir.AxisListType.X, op=mybir.AluOpType.min)
        nc.vector.tensor_add(d2b[:], d2a[:], sqloc_sb[:])      # + sq_i
        nc.vector.tensor_scalar_max(d2a[:], d2b[:], 1e-12)     # NaN guard
        nc.scalar.activation(lnout[:], d2a[:],
                             mybir.ActivationFunctionType.Ln)
        nc.sync.dma_start(out[:], lnout[:])
    return nc


_ENGINE_ATTR = {
    "PE": "tensor", "DVE": "vector", "ACT": "scalar",
    "Pool": "gpsimd", "SP": "sync",
}


def _split_waits(nc: bass.Bass) -> bass.Bass:
    """Walrus in this toolchain accepts at most ONE attached sync wait per
    instruction. Hoist extra waits onto fresh same-engine NOPs inserted
    immediately before the over-subscribed instruction (identical ordering
    semantics: the engine executes nop(wait) then inst(wait))."""
    import bass_rust

    f = nc.m.functions[0]

    def make_nop(engine) -> object:
        name = str(engine).split(".")[-1]
        eng = getattr(nc, _ENGINE_ATTR.get(name, "vector"))
        bi = eng.nop()
        inst = bi.ins
        for b in f.blocks:
            for idx in range(len(b.instructions) - 1, -1, -1):
                if b.instructions[idx] is inst:
                    lst = list(b.instructions)
                    del lst[idx]
                    b.instructions = lst
                    return inst
        raise RuntimeError("fresh nop not found in any block")

    for b in f.blocks:
        changed = False
        out = []
        for inst in b.instructions:
            si = getattr(inst, "sync_info", None)
            w = list(si.on_wait) if (si is not None and si.on_wait) else []
            if len(w) > 1:
                for extra in w[:-1]:
                    nop = make_nop(inst.engine)
                    nop.sync_info = bass_rust.SyncInfo(on_wait=[extra],
                                                       on_update=[])
                    out.append(nop)
                si.on_wait = [w[-1]]
                changed = True
            out.append(inst)
        if changed:
            b.instructions = out
    return nc


def _prep_common(x: np.ndarray):
    x = np.ascontiguousarray(x, dtype=np.float32)
    sq = (x.astype(np.float64) ** 2).sum(axis=1).astype(np.float32)   # (B,)
    xt = np.ascontiguousarray(x.T)                                    # (512, B)
    return x, sq, xt


def _prep_inputs_act(x: np.ndarray) -> list[dict[str, np.ndarray]]:
    x, sq, xt = _prep_common(x)
    in_maps = []
    for c in range(N_CORES):
        rot_b = np.roll(xt, -RPC * c, axis=1).astype(ml_dtypes.bfloat16)
        xt_c = np.ascontiguousarray(rot_b.reshape(K_TILES, 128, B))
        sq_rot = np.roll(sq, -RPC * c)
        sqh_c = np.ascontiguousarray(
            np.broadcast_to(-0.5 * sq_rot[None, :], (128, B))).astype(np.float32)
        sqhd_c = np.empty((128, M_TILES * 512), np.float32)
        for m in range(M_TILES):
            blk = sqh_c[:, (m // 4) * 512:(m // 4 + 1) * 512].copy()
            o = 128 * (m % 4)
            blk[np.arange(128), o + np.arange(128)] = -BIG
            sqhd_c[:, m * 512:(m + 1) * 512] = blk
        sqloc_c = np.ascontiguousarray(
            sq[RPC * c:RPC * (c + 1)].reshape(M_TILES, 128).T)        # (128, 8)
        in_maps.append({"xt": xt_c, "sqh": sqh_c, "sqhd": sqhd_c,
                        "sqloc": sqloc_c})
    return in_maps


def _prep_inputs_mm5(x: np.ndarray) -> list[dict[str, np.ndarray]]:
    x, sq, xt = _prep_common(x)
    sq_hi = sq.astype(ml_dtypes.bfloat16)
    sq_lo = (sq - sq_hi.astype(np.float32)).astype(ml_dtypes.bfloat16)
    ones2 = np.ones((2, 128), ml_dtypes.bfloat16)
    diagc = np.zeros((128, 128), np.float32)
    np.fill_diagonal(diagc, BIG)
    in_maps = []
    for c in range(N_CORES):
        rot_b = np.roll(xt, -RPC * c, axis=1).astype(ml_dtypes.bfloat16)
        xt_c = np.ascontiguousarray(
            rot_b.reshape(K_TILES, 128, N_TILES, 512).transpose(0, 2, 1, 3))
        xlt2_c = np.ascontiguousarray(rot_b[:, :RPC].reshape(K_TILES, 128, RPC)
                                      ) * np.float32(-2.0)
        sqr_c = np.stack([np.roll(sq_hi, -RPC * c), np.roll(sq_lo, -RPC * c)])
        sqloc_c = np.ascontiguousarray(
            sq[RPC * c:RPC * (c + 1)].reshape(M_TILES, 128).T)
        in_maps.append({"xt": xt_c, "xlt2": xlt2_c.astype(ml_dtypes.bfloat16),
                        "sqr": sqr_c, "ones2": ones2, "diagc": diagc,
                        "sqloc": sqloc_c})
    return in_maps


def _run(inputs: np.ndarray, trace: bool = False, variant: str | None = None):
    v = variant or VARIANT
    if v == "act_init":
        nc, in_maps = _build_act_init(), _prep_inputs_act(inputs)
    else:
        nc, in_maps = _build_mm5(), _prep_inputs_mm5(inputs)
    _split_waits(nc)
    res = run_bass_kernel_spmd(nc, in_maps, list(range(N_CORES)), trace=trace)
    lnd2 = np.stack([res.results[c]["lnd2"] for c in range(N_CORES)])
    loss = np.float32(-0.5 * np.mean(lnd2.astype(np.float64)))
    return np.asarray(loss, dtype=np.float32), res


def kernel(inputs: np.ndarray) -> np.ndarray:
    out, _ = _run(inputs, trace=False)
    return out


# revision 7
# speedup vs baseline: 1.2024x; 1.2024x over previous
"""KoLeo loss kernel for Trainium2 (8 NeuronCores, Bass/Tile).

loss = mean_i( -log( min_{j != i} ||x_i - x_j|| + eps ) ) for x: (8192, 512) f32.

Data-parallel over rows, 1024 rows/core. Per core, for each of 8x16 (m, n)
tiles of the local Gram block:
  variant "act_init":
    PSUM tile is initialized by a ScalarE copy with w0 = -0.5*sq_j (diag tile:
    -BIG at the diagonal), then 4 bf16 matmuls accumulate +g on top
    (start=False rides the has_written bits, which engine writes preserve;
    8 warm-up matmuls pin them set). Then one VectorE row-max per tile:
      max_j (g - 0.5 sq_j)  ->  d2_i = sq_i - 2*max  == min_{j!=i} d2_ij.
  variant "mm5":
    4 bf16 matmuls of -2g (pre-scaled lhsT) + a 5th K=2 matmul adding
    ones x [sq_hi; sq_lo], then a VectorE row-min; the diagonal is masked by
    adding +BIG to the 128-wide diag band in separate small ops.

Each core works on a column-rotated copy of X^T (rotated by 1024*core) so the
diagonal lands at rotated columns [128m, 128m+128) on every core -> identical
SPMD program; only input data differs. Device outputs ln(d2min) per local
row; host does -0.5 * mean.
"""

import os
from contextlib import ExitStack

import numpy as np

os.environ.setdefault("JAX_PLATFORMS", "axon,cpu")

import ml_dtypes  # noqa: E402

import concourse.bass as bass  # noqa: E402
import concourse.tile as tile  # noqa: E402
from concourse import mybir  # noqa: E402
from concourse.bass_utils import run_bass_kernel_spmd  # noqa: E402

B, D = 8192, 512
N_CORES = 8
RPC = B // N_CORES            # rows per core: 1024
M_TILES = RPC // 128          # 8
N_TILES = B // 512            # 16
K_TILES = D // 128            # 4
BIG = 1.0e30
F32 = mybir.dt.float32
BF16 = mybir.dt.bfloat16

VARIANT = os.environ.get("KOLEO_VARIANT", "act_init")


def _build_act_init() -> bass.Bass:
    nc = bass.Bass("TRN2", target_bir_lowering=False, debug=False,
                   num_devices=N_CORES)
    xt = nc.dram_tensor("xt", [K_TILES, 128, B], BF16,
                        kind="ExternalInput").ap()
    sqh = nc.dram_tensor("sqh", [128, B], F32, kind="ExternalInput").ap()
    sqhd = nc.dram_tensor("sqhd", [128, M_TILES * 512], F32,
                          kind="ExternalInput").ap()
    sqloc = nc.dram_tensor("sqloc", [128, M_TILES], F32,
                           kind="ExternalInput").ap()
    out = nc.dram_tensor("lnd2", [128, M_TILES], F32,
                         kind="ExternalOutput").ap()

    with tile.TileContext(nc) as tc, ExitStack() as ctx:
        big = ctx.enter_context(tc.tile_pool(name="big", bufs=1))
        small = ctx.enter_context(tc.tile_pool(name="small", bufs=1))
        pp = ctx.enter_context(tc.tile_pool(name="pp", bufs=8, space="PSUM"))

        xt_sb = [big.tile([128, B], BF16, tag=f"xt_{k}", bufs=1,
                          name=f"xt_{k}") for k in range(K_TILES)]
        sqh_sb = big.tile([128, B], F32, tag="sqh", bufs=1)
        sqhd_sb = big.tile([128, M_TILES * 512], F32, tag="sqhd", bufs=1)
        sqloc_sb = small.tile([128, M_TILES], F32, tag="sqloc", bufs=1)
        maxb = small.tile([128, M_TILES * N_TILES], F32, tag="maxb", bufs=1)
        d2a = small.tile([128, M_TILES], F32, tag="d2a", bufs=1)
        d2b = small.tile([128, M_TILES], F32, tag="d2b", bufs=1)
        lnout = small.tile([128, M_TILES], F32, tag="lnout", bufs=1)
        scr_a = small.tile([1, 1], F32, tag="scr_a", bufs=1)
        scr_b = small.tile([1, 1], F32, tag="scr_b", bufs=1)
        scr_c = small.tile([1, 1], F32, tag="scr_c", bufs=1)
        scr2 = small.tile([1, 1], F32, tag="scr2", bufs=1)

        QCH = B // 4              # 2048-col DMA chunks, in consumption order
        HH = M_TILES * 512 // 2
        for q in range(4):
            for k in range(K_TILES):
                nc.sync.dma_start(xt_sb[k][:, q * QCH:(q + 1) * QCH],
                                  xt[k, :, q * QCH:(q + 1) * QCH])
            nc.sync.dma_start(sqh_sb[:, q * QCH:(q + 1) * QCH],
                              sqh[:, q * QCH:(q + 1) * QCH])
            if q < 2:
                nc.sync.dma_start(sqhd_sb[:, q * HH:(q + 1) * HH],
                                  sqhd[:, q * HH:(q + 1) * HH])
        nc.sync.dma_start(sqloc_sb[:], sqloc[:])

        # Wait-absorbers: dedicated first-touches so hot-loop instructions
        # need at most one attached sync wait (walrus AC/DVE struct limit).
        nc.scalar.activation(scr_a[:], sqh_sb[0:1, 0:1],
                             mybir.ActivationFunctionType.Copy)
        nc.scalar.activation(scr_b[:], sqhd_sb[0:1, 0:1],
                             mybir.ActivationFunctionType.Copy)
        nc.scalar.activation(scr_c[:], scr_a[:],
                             mybir.ActivationFunctionType.Ln)  # const-bias load
        nc.vector.tensor_copy(scr2[:], sqloc_sb[0:1, 0:1])

        # Warm-up matmuls: pin has_written on every element of all 8 banks so
        # the start=False groups below accumulate onto ScalarE-written init.
        for b_ in range(8):
            ps = pp.tile([128, 512], F32, tag="ps", bufs=8, name=f"warm_{b_}")
            nc.tensor.matmul(ps[:], xt_sb[0][:, 0:128], xt_sb[0][:, 0:512],
                             start=True, stop=True)

        for n in range(N_TILES):
            for m in range(M_TILES):
                ps = pp.tile([128, 512], F32, tag="ps", bufs=8,
                             name=f"ps_{n}_{m}")
                nd = m // 4               # n-tile holding the diagonal
                init = (sqhd_sb[:, m * 512:(m + 1) * 512] if n == nd
                        else sqh_sb[:, n * 512:(n + 1) * 512])
                nc.scalar.activation(ps[:], init,
                                     mybir.ActivationFunctionType.Copy)
                for k in range(K_TILES):
                    nc.tensor.matmul(
                        ps[:],
                        xt_sb[k][:, m * 128:(m + 1) * 128],  # lhsT: local cols
                        xt_sb[k][:, n * 512:(n + 1) * 512],
                        start=False,
                        stop=(k == K_TILES - 1),
                        skip_group_check=True,
                    )
                col = m * N_TILES + n
                nc.vector.tensor_reduce(
                    out=maxb[:, col:col + 1], in_=ps[:],
                    axis=mybir.AxisListType.X, op=mybir.AluOpType.max)

        for m in range(M_TILES):
            nc.vector.tensor_reduce(
                out=d2a[:, m:m + 1],
                in_=maxb[:, m * N_TILES:(m + 1) * N_TILES],
                axis=mybir.AxisListType.X, op=mybir.AluOpType.max)
        nc.vector.tensor_scalar_mul(d2b[:], d2a[:], -2.0)
        nc.vector.tensor_add(d2a[:], d2b[:], sqloc_sb[:])      # + sq_i
        nc.vector.tensor_scalar_max(d2b[:], d2a[:], 1e-12)     # NaN guard
        nc.scalar.activation(lnout[:], d2b[:],
                             mybir.ActivationFunctionType.Ln)
        nc.sync.dma_start(out[:], lnout[:])
    return nc


def _build_mm5() -> bass.Bass:
    nc = bass.Bass("TRN2", target_bir_lowering=False, debug=False,
                   num_devices=N_CORES)
    xt = nc.dram_tensor("xt", [K_TILES, N_TILES, 128, 512], BF16,
                        kind="ExternalInput").ap()
    xlt2 = nc.dram_tensor("xlt2", [K_TILES, 128, RPC], BF16,
                          kind="ExternalInput").ap()
    sqr = nc.dram_tensor("sqr", [2, B], BF16, kind="ExternalInput").ap()
    ones2 = nc.dram_tensor("ones2", [2, 128], BF16, kind="ExternalInput").ap()
    diagc = nc.dram_tensor("diagc", [128, 128], F32, kind="ExternalInput").ap()
    sqloc = nc.dram_tensor("sqloc", [128, M_TILES], F32,
                           kind="ExternalInput").ap()
    out = nc.dram_tensor("lnd2", [128, M_TILES], F32,
                         kind="ExternalOutput").ap()

    NCOL = N_TILES + 2  # per-m minb cols: 15 regular + up to 3 diag pieces
    with tile.TileContext(nc) as tc, ExitStack() as ctx:
        xt_pool = ctx.enter_context(tc.tile_pool(name="xtp", bufs=K_TILES * N_TILES))
        aux = ctx.enter_context(tc.tile_pool(name="aux", bufs=1))
        band_pool = ctx.enter_context(tc.tile_pool(name="bandp", bufs=2))
        pp = ctx.enter_context(tc.tile_pool(name="pp", bufs=8, space="PSUM"))

        xt_sb = [[xt_pool.tile([128, 512], BF16, tag="xt",
                               bufs=K_TILES * N_TILES, name=f"xt_{k}_{n}")
                  for n in range(N_TILES)] for k in range(K_TILES)]
        xlt2_sb = [aux.tile([128, RPC], BF16, tag=f"xlt2_{k}", bufs=1,
                            name=f"xlt2_{k}") for k in range(K_TILES)]
        sqr_sb = aux.tile([2, B], BF16, tag="sqr", bufs=1)
        ones2_sb = aux.tile([2, 128], BF16, tag="ones2", bufs=1)
        diagc_sb = aux.tile([128, 128], F32, tag="diagc", bufs=1)
        sqloc_sb = aux.tile([128, M_TILES], F32, tag="sqloc", bufs=1)
        minb = aux.tile([128, M_TILES * NCOL], F32, tag="minb", bufs=1)
        d2a = aux.tile([128, M_TILES], F32, tag="d2a", bufs=1)
        d2b = aux.tile([128, M_TILES], F32, tag="d2b", bufs=1)
        lnout = aux.tile([128, M_TILES], F32, tag="lnout", bufs=1)

        nc.sync.dma_start(sqloc_sb[:], sqloc[:])
        nc.sync.dma_start(sqr_sb[:], sqr[:])
        nc.sync.dma_start(ones2_sb[:], ones2[:])
        nc.sync.dma_start(diagc_sb[:], diagc[:])
        for k in range(K_TILES):
            nc.sync.dma_start(xlt2_sb[k][:], xlt2[k])
        for n in range(N_TILES):
            for k in range(K_TILES):
                nc.sync.dma_start(xt_sb[k][n][:], xt[k, n])

        nc.vector.memset(minb[:], BIG)

        for n in range(N_TILES):
            for m in range(M_TILES):
                ps = pp.tile([128, 512], F32, tag="ps", bufs=8,
                             name=f"ps_{n}_{m}")
                for k in range(K_TILES):
                    nc.tensor.matmul(
                        ps[:],
                        xlt2_sb[k][:, m * 128:(m + 1) * 128],
                        xt_sb[k][n][:],
                        start=(k == 0),
                        stop=False,
                    )
                nc.tensor.matmul(                      # += ones (x) sq rows
                    ps[:],
                    ones2_sb[:],
                    sqr_sb[:, n * 512:(n + 1) * 512],
                    start=False,
                    stop=True,
                )
                nd = m // 4
                o = 128 * (m % 4)
                cb = m * NCOL
                if n != nd:
                    col = cb + n - (1 if n > nd else 0)
                    nc.vector.tensor_reduce(
                        out=minb[:, col:col + 1], in_=ps[:],
                        axis=mybir.AxisListType.X, op=mybir.AluOpType.min)
                else:
                    band = band_pool.tile([128, 128], F32, tag="band", bufs=2,
                                          name=f"band_{m}")
                    nc.vector.tensor_add(band[:], ps[:, o:o + 128], diagc_sb[:])
                    nc.vector.tensor_reduce(
                        out=minb[:, cb + 15:cb + 16], in_=band[:],
                        axis=mybir.AxisListType.X, op=mybir.AluOpType.min)
                    if o > 0:
                        nc.vector.tensor_reduce(
                            out=minb[:, cb + 16:cb + 17], in_=ps[:, 0:o],
                            axis=mybir.AxisListType.X, op=mybir.AluOpType.min)
                    if o < 384:
                        nc.vector.tensor_reduce(
                            out=minb[:, cb + 17:cb + 18],
                            in_=ps[:, o + 128:512],
                            axis=mybir.AxisListType.X, op=mybir.AluOpType.min)

        for m in range(M_TILES):
            nc.vector.tensor_reduce(
                out=d2a[:, m:m + 1],
                in_=minb[:, m * NCOL:(m + 1) * NCOL],
                axis=mybir.AxisListType.X, op=mybir.AluOpType.min)
        nc.vector.tensor_add(d2b[:], d2a[:], sqloc_sb[:])      # + sq_i
        nc.vector.tensor_scalar_max(d2a[:], d2b[:], 1e-12)     # NaN guard
        nc.scalar.activation(lnout[:], d2a[:],
                             mybir.ActivationFunctionType.Ln)
        nc.sync.dma_start(out[:], lnout[:])
    return nc


_ENGINE_ATTR = {
    "PE": "tensor", "DVE": "vector", "ACT": "scalar",
    "Pool": "gpsimd", "SP": "sync",
}


def _split_waits(nc: bass.Bass) -> bass.Bass:
    """Walrus in this toolchain accepts at most ONE attached sync wait per
    instruction. Hoist extra waits onto fresh same-engine NOPs inserted
    immediately before the over-subscribed instruction (identical ordering
    semantics: the engine executes nop(wait) then inst(wait))."""
    import bass_rust

    f = nc.m.functions[0]

    def make_nop(engine) -> object:
        name = str(engine).split(".")[-1]
        eng = getattr(nc, _ENGINE_ATTR.get(name, "vector"))
        bi = eng.nop()
        inst = bi.ins
        for b in f.blocks:
            for idx in range(len(b.instructions) - 1, -1, -1):
                if b.instructions[idx] is inst:
                    lst = list(b.instructions)
                    del lst[idx]
                    b.instructions = lst
                    return inst
        raise RuntimeError("fresh nop not found in any block")

    for b in f.blocks:
        changed = False
        out = []
        for inst in b.instructions:
            si = getattr(inst, "sync_info", None)
            w = list(si.on_wait) if (si is not None and si.on_wait) else []
            if len(w) > 1:
                for extra in w[:-1]:
                    nop = make_nop(inst.engine)
                    nop.sync_info = bass_rust.SyncInfo(on_wait=[extra],
                                                       on_update=[])
                    out.append(nop)
                si.on_wait = [w[-1]]
                changed = True
            out.append(inst)
        if changed:
            b.instructions = out
    return nc


def _prep_common(x: np.ndarray):
    x = np.ascontiguousarray(x, dtype=np.float32)
    sq = (x.astype(np.float64) ** 2).sum(axis=1).astype(np.float32)   # (B,)
    xt = np.ascontiguousarray(x.T)                                    # (512, B)
    return x, sq, xt


def _prep_inputs_act(x: np.ndarray) -> list[dict[str, np.ndarray]]:
    x, sq, xt = _prep_common(x)
    in_maps = []
    for c in range(N_CORES):
        rot_b = np.roll(xt, -RPC * c, axis=1).astype(ml_dtypes.bfloat16)
        xt_c = np.ascontiguousarray(rot_b.reshape(K_TILES, 128, B))
        sq_rot = np.roll(sq, -RPC * c)
        sqh_c = np.ascontiguousarray(
            np.broadcast_to(-0.5 * sq_rot[None, :], (128, B))).astype(np.float32)
        sqhd_c = np.empty((128, M_TILES * 512), np.float32)
        for m in range(M_TILES):
            blk = sqh_c[:, (m // 4) * 512:(m // 4 + 1) * 512].copy()
            o = 128 * (m % 4)
            blk[np.arange(128), o + np.arange(128)] = -BIG
            sqhd_c[:, m * 512:(m + 1) * 512] = blk
        sqloc_c = np.ascontiguousarray(
            sq[RPC * c:RPC * (c + 1)].reshape(M_TILES, 128).T)        # (128, 8)
        in_maps.append({"xt": xt_c, "sqh": sqh_c, "sqhd": sqhd_c,
                        "sqloc": sqloc_c})
    return in_maps


def _prep_inputs_mm5(x: np.ndarray) -> list[dict[str, np.ndarray]]:
    x, sq, xt = _prep_common(x)
    sq_hi = sq.astype(ml_dtypes.bfloat16)
    sq_lo = (sq - sq_hi.astype(np.float32)).astype(ml_dtypes.bfloat16)
    ones2 = np.ones((2, 128), ml_dtypes.bfloat16)
    diagc = np.zeros((128, 128), np.float32)
    np.fill_diagonal(diagc, BIG)
    in_maps = []
    for c in range(N_CORES):
        rot_b = np.roll(xt, -RPC * c, axis=1).astype(ml_dtypes.bfloat16)
        xt_c = np.ascontiguousarray(
            rot_b.reshape(K_TILES, 128, N_TILES, 512).transpose(0, 2, 1, 3))
        xlt2_c = np.ascontiguousarray(rot_b[:, :RPC].reshape(K_TILES, 128, RPC)
                                      ) * np.float32(-2.0)
        sqr_c = np.stack([np.roll(sq_hi, -RPC * c), np.roll(sq_lo, -RPC * c)])
        sqloc_c = np.ascontiguousarray(
            sq[RPC * c:RPC * (c + 1)].reshape(M_TILES, 128).T)
        in_maps.append({"xt": xt_c, "xlt2": xlt2_c.astype(ml_dtypes.bfloat16),
                        "sqr": sqr_c, "ones2": ones2, "diagc": diagc,
                        "sqloc": sqloc_c})
    return in_maps


def _run(inputs: np.ndarray, trace: bool = False, variant: str | None = None):
    v = variant or VARIANT
    if v == "act_init":
        nc, in_maps = _build_act_init(), _prep_inputs_act(inputs)
    else:
        nc, in_maps = _build_mm5(), _prep_inputs_mm5(inputs)
    _split_waits(nc)
    res = run_bass_kernel_spmd(nc, in_maps, list(range(N_CORES)), trace=trace)
    lnd2 = np.stack([res.results[c]["lnd2"] for c in range(N_CORES)])
    loss = np.float32(-0.5 * np.mean(lnd2.astype(np.float64)))
    return np.asarray(loss, dtype=np.float32), res


def kernel(inputs: np.ndarray) -> np.ndarray:
    out, _ = _run(inputs, trace=False)
    return out


# revision 8
# speedup vs baseline: 1.2227x; 1.0168x over previous
"""KoLeo loss kernel for Trainium2 (8 NeuronCores, Bass/Tile).

loss = mean_i( -log( min_{j != i} ||x_i - x_j|| + eps ) ) for x: (8192, 512) f32.

Data-parallel over rows, 1024 rows/core. Per core, for each of 8x16 (m, n)
tiles of the local Gram block:
  variant "act_init":
    PSUM tile is initialized by a ScalarE copy with w0 = -0.5*sq_j (diag tile:
    -BIG at the diagonal), then 4 bf16 matmuls accumulate +g on top
    (start=False rides the has_written bits, which engine writes preserve;
    8 warm-up matmuls pin them set). Then one VectorE row-max per tile:
      max_j (g - 0.5 sq_j)  ->  d2_i = sq_i - 2*max  == min_{j!=i} d2_ij.
  variant "mm5":
    4 bf16 matmuls of -2g (pre-scaled lhsT) + a 5th K=2 matmul adding
    ones x [sq_hi; sq_lo], then a VectorE row-min; the diagonal is masked by
    adding +BIG to the 128-wide diag band in separate small ops.

Each core works on a column-rotated copy of X^T (rotated by 1024*core) so the
diagonal lands at rotated columns [128m, 128m+128) on every core -> identical
SPMD program; only input data differs. Device outputs ln(d2min) per local
row; host does -0.5 * mean.
"""

import os
from contextlib import ExitStack

import numpy as np

os.environ.setdefault("JAX_PLATFORMS", "axon,cpu")

import ml_dtypes  # noqa: E402

import concourse.bass as bass  # noqa: E402
import concourse.tile as tile  # noqa: E402
from concourse import mybir  # noqa: E402
from concourse.bass_utils import run_bass_kernel_spmd  # noqa: E402

B, D = 8192, 512
N_CORES = 8
RPC = B // N_CORES            # rows per core: 1024
M_TILES = RPC // 128          # 8
N_TILES = B // 512            # 16
K_TILES = D // 128            # 4
BIG = 1.0e30
F32 = mybir.dt.float32
BF16 = mybir.dt.bfloat16

VARIANT = os.environ.get("KOLEO_VARIANT", "act_init")


def _build_act_init() -> bass.Bass:
    nc = bass.Bass("TRN2", target_bir_lowering=False, debug=False,
                   num_devices=N_CORES)
    xt = nc.dram_tensor("xt", [K_TILES, 128, B], BF16,
                        kind="ExternalInput").ap()
    sqh = nc.dram_tensor("sqh", [128, B], F32, kind="ExternalInput").ap()
    sqhd = nc.dram_tensor("sqhd", [128, M_TILES * 512], F32,
                          kind="ExternalInput").ap()
    sqloc = nc.dram_tensor("sqloc", [128, M_TILES], F32,
                           kind="ExternalInput").ap()
    out = nc.dram_tensor("lnd2", [128, M_TILES], F32,
                         kind="ExternalOutput").ap()

    with tile.TileContext(nc) as tc, ExitStack() as ctx:
        big = ctx.enter_context(tc.tile_pool(name="big", bufs=1))
        small = ctx.enter_context(tc.tile_pool(name="small", bufs=1))
        pp = ctx.enter_context(tc.tile_pool(name="pp", bufs=8, space="PSUM"))

        xt_sb = [big.tile([128, B], BF16, tag=f"xt_{k}", bufs=1,
                          name=f"xt_{k}") for k in range(K_TILES)]
        sqh_sb = big.tile([128, B], F32, tag="sqh", bufs=1)
        sqhd_sb = big.tile([128, M_TILES * 512], F32, tag="sqhd", bufs=1)
        sqloc_sb = small.tile([128, M_TILES], F32, tag="sqloc", bufs=1)
        maxb = small.tile([128, M_TILES * N_TILES], F32, tag="maxb", bufs=1)
        d2a = small.tile([128, M_TILES], F32, tag="d2a", bufs=1)
        d2b = small.tile([128, M_TILES], F32, tag="d2b", bufs=1)
        lnout = small.tile([128, M_TILES], F32, tag="lnout", bufs=1)
        scr_a = small.tile([1, 1], F32, tag="scr_a", bufs=1)
        scr_b = small.tile([1, 1], F32, tag="scr_b", bufs=1)
        scr_c = small.tile([1, 1], F32, tag="scr_c", bufs=1)
        scr2 = small.tile([1, 1], F32, tag="scr2", bufs=1)

        QCH = B // 8              # 1024-col DMA chunks, in consumption order
        HH = M_TILES * 512 // 4
        for q in range(8):
            for k in range(K_TILES):
                nc.sync.dma_start(xt_sb[k][:, q * QCH:(q + 1) * QCH],
                                  xt[k, :, q * QCH:(q + 1) * QCH])
            nc.sync.dma_start(sqh_sb[:, q * QCH:(q + 1) * QCH],
                              sqh[:, q * QCH:(q + 1) * QCH])
            if q < 4:
                nc.sync.dma_start(sqhd_sb[:, q * HH:(q + 1) * HH],
                                  sqhd[:, q * HH:(q + 1) * HH])
        nc.sync.dma_start(sqloc_sb[:], sqloc[:])

        # Wait-absorbers: dedicated first-touches so hot-loop instructions
        # need at most one attached sync wait (walrus AC/DVE struct limit).
        nc.scalar.activation(scr_a[:], sqh_sb[0:1, 0:1],
                             mybir.ActivationFunctionType.Copy)
        nc.scalar.activation(scr_b[:], sqhd_sb[0:1, 0:1],
                             mybir.ActivationFunctionType.Copy)
        nc.scalar.activation(scr_c[:], scr_a[:],
                             mybir.ActivationFunctionType.Ln)  # const-bias load
        nc.vector.tensor_copy(scr2[:], sqloc_sb[0:1, 0:1])

        # Warm-up matmuls: pin has_written on every element of all 8 banks so
        # the start=False groups below accumulate onto ScalarE-written init.
        for b_ in range(8):
            ps = pp.tile([128, 512], F32, tag="ps", bufs=8, name=f"warm_{b_}")
            nc.tensor.matmul(ps[:], xt_sb[0][:, 0:128], xt_sb[0][:, 0:512],
                             start=True, stop=True)

        for n in range(N_TILES):
            for m in range(M_TILES):
                ps = pp.tile([128, 512], F32, tag="ps", bufs=8,
                             name=f"ps_{n}_{m}")
                nd = m // 4               # n-tile holding the diagonal
                init = (sqhd_sb[:, m * 512:(m + 1) * 512] if n == nd
                        else sqh_sb[:, n * 512:(n + 1) * 512])
                nc.scalar.activation(ps[:], init,
                                     mybir.ActivationFunctionType.Copy)
                for k in range(K_TILES):
                    nc.tensor.matmul(
                        ps[:],
                        xt_sb[k][:, m * 128:(m + 1) * 128],  # lhsT: local cols
                        xt_sb[k][:, n * 512:(n + 1) * 512],
                        start=False,
                        stop=(k == K_TILES - 1),
                        skip_group_check=True,
                    )
                col = m * N_TILES + n
                nc.vector.tensor_reduce(
                    out=maxb[:, col:col + 1], in_=ps[:],
                    axis=mybir.AxisListType.X, op=mybir.AluOpType.max)

        for m in range(M_TILES):
            nc.vector.tensor_reduce(
                out=d2a[:, m:m + 1],
                in_=maxb[:, m * N_TILES:(m + 1) * N_TILES],
                axis=mybir.AxisListType.X, op=mybir.AluOpType.max)
        nc.vector.tensor_scalar_mul(d2b[:], d2a[:], -2.0)
        nc.vector.tensor_add(d2a[:], d2b[:], sqloc_sb[:])      # + sq_i
        nc.vector.tensor_scalar_max(d2b[:], d2a[:], 1e-12)     # NaN guard
        nc.scalar.activation(lnout[:], d2b[:],
                             mybir.ActivationFunctionType.Ln)
        nc.sync.dma_start(out[:], lnout[:])
    return nc


def _build_mm5() -> bass.Bass:
    nc = bass.Bass("TRN2", target_bir_lowering=False, debug=False,
                   num_devices=N_CORES)
    xt = nc.dram_tensor("xt", [K_TILES, N_TILES, 128, 512], BF16,
                        kind="ExternalInput").ap()
    xlt2 = nc.dram_tensor("xlt2", [K_TILES, 128, RPC], BF16,
                          kind="ExternalInput").ap()
    sqr = nc.dram_tensor("sqr", [2, B], BF16, kind="ExternalInput").ap()
    ones2 = nc.dram_tensor("ones2", [2, 128], BF16, kind="ExternalInput").ap()
    diagc = nc.dram_tensor("diagc", [128, 128], F32, kind="ExternalInput").ap()
    sqloc = nc.dram_tensor("sqloc", [128, M_TILES], F32,
                           kind="ExternalInput").ap()
    out = nc.dram_tensor("lnd2", [128, M_TILES], F32,
                         kind="ExternalOutput").ap()

    NCOL = N_TILES + 2  # per-m minb cols: 15 regular + up to 3 diag pieces
    with tile.TileContext(nc) as tc, ExitStack() as ctx:
        xt_pool = ctx.enter_context(tc.tile_pool(name="xtp", bufs=K_TILES * N_TILES))
        aux = ctx.enter_context(tc.tile_pool(name="aux", bufs=1))
        band_pool = ctx.enter_context(tc.tile_pool(name="bandp", bufs=2))
        pp = ctx.enter_context(tc.tile_pool(name="pp", bufs=8, space="PSUM"))

        xt_sb = [[xt_pool.tile([128, 512], BF16, tag="xt",
                               bufs=K_TILES * N_TILES, name=f"xt_{k}_{n}")
                  for n in range(N_TILES)] for k in range(K_TILES)]
        xlt2_sb = [aux.tile([128, RPC], BF16, tag=f"xlt2_{k}", bufs=1,
                            name=f"xlt2_{k}") for k in range(K_TILES)]
        sqr_sb = aux.tile([2, B], BF16, tag="sqr", bufs=1)
        ones2_sb = aux.tile([2, 128], BF16, tag="ones2", bufs=1)
        diagc_sb = aux.tile([128, 128], F32, tag="diagc", bufs=1)
        sqloc_sb = aux.tile([128, M_TILES], F32, tag="sqloc", bufs=1)
        minb = aux.tile([128, M_TILES * NCOL], F32, tag="minb", bufs=1)
        d2a = aux.tile([128, M_TILES], F32, tag="d2a", bufs=1)
        d2b = aux.tile([128, M_TILES], F32, tag="d2b", bufs=1)
        lnout = aux.tile([128, M_TILES], F32, tag="lnout", bufs=1)

        nc.sync.dma_start(sqloc_sb[:], sqloc[:])
        nc.sync.dma_start(sqr_sb[:], sqr[:])
        nc.sync.dma_start(ones2_sb[:], ones2[:])
        nc.sync.dma_start(diagc_sb[:], diagc[:])
        for k in range(K_TILES):
            nc.sync.dma_start(xlt2_sb[k][:], xlt2[k])
        for n in range(N_TILES):
            for k in range(K_TILES):
                nc.sync.dma_start(xt_sb[k][n][:], xt[k, n])

        nc.vector.memset(minb[:], BIG)

        for n in range(N_TILES):
            for m in range(M_TILES):
                ps = pp.tile([128, 512], F32, tag="ps", bufs=8,
                             name=f"ps_{n}_{m}")
                for k in range(K_TILES):
                    nc.tensor.matmul(
                        ps[:],
                        xlt2_sb[k][:, m * 128:(m + 1) * 128],
                        xt_sb[k][n][:],
                        start=(k == 0),
                        stop=False,
                    )
                nc.tensor.matmul(                      # += ones (x) sq rows
                    ps[:],
                    ones2_sb[:],
                    sqr_sb[:, n * 512:(n + 1) * 512],
                    start=False,
                    stop=True,
                )
                nd = m // 4
                o = 128 * (m % 4)
                cb = m * NCOL
                if n != nd:
                    col = cb + n - (1 if n > nd else 0)
                    nc.vector.tensor_reduce(
                        out=minb[:, col:col + 1], in_=ps[:],
                        axis=mybir.AxisListType.X, op=mybir.AluOpType.min)
                else:
                    band = band_pool.tile([128, 128], F32, tag="band", bufs=2,
                                          name=f"band_{m}")
                    nc.vector.tensor_add(band[:], ps[:, o:o + 128], diagc_sb[:])
                    nc.vector.tensor_reduce(
                        out=minb[:, cb + 15:cb + 16], in_=band[:],
                        axis=mybir.AxisListType.X, op=mybir.AluOpType.min)
                    if o > 0:
                        nc.vector.tensor_reduce(
                            out=minb[:, cb + 16:cb + 17], in_=ps[:, 0:o],
                            axis=mybir.AxisListType.X, op=mybir.AluOpType.min)
                    if o < 384:
                        nc.vector.tensor_reduce(
                            out=minb[:, cb + 17:cb + 18],
                            in_=ps[:, o + 128:512],
                            axis=mybir.AxisListType.X, op=mybir.AluOpType.min)

        for m in range(M_TILES):
            nc.vector.tensor_reduce(
                out=d2a[:, m:m + 1],
                in_=minb[:, m * NCOL:(m + 1) * NCOL],
                axis=mybir.AxisListType.X, op=mybir.AluOpType.min)
        nc.vector.tensor_add(d2b[:], d2a[:], sqloc_sb[:])      # + sq_i
        nc.vector.tensor_scalar_max(d2a[:], d2b[:], 1e-12)     # NaN guard
        nc.scalar.activation(lnout[:], d2a[:],
                             mybir.ActivationFunctionType.Ln)
        nc.sync.dma_start(out[:], lnout[:])
    return nc


_ENGINE_ATTR = {
    "PE": "tensor", "DVE": "vector", "ACT": "scalar",
    "Pool": "gpsimd", "SP": "sync",
}


def _split_waits(nc: bass.Bass) -> bass.Bass:
    """Walrus in this toolchain accepts at most ONE attached sync wait per
    instruction. Hoist extra waits onto fresh same-engine NOPs inserted
    immediately before the over-subscribed instruction (identical ordering
    semantics: the engine executes nop(wait) then inst(wait))."""
    import bass_rust

    f = nc.m.functions[0]

    def make_nop(engine) -> object:
        name = str(engine).split(".")[-1]
        eng = getattr(nc, _ENGINE_ATTR.get(name, "vector"))
        bi = eng.nop()
        inst = bi.ins
        for b in f.blocks:
            for idx in range(len(b.instructions) - 1, -1, -1):
                if b.instructions[idx] is inst:
                    lst = list(b.instructions)
                    del lst[idx]
                    b.instructions = lst
                    return inst
        raise RuntimeError("fresh nop not found in any block")

    for b in f.blocks:
        changed = False
        out = []
        for inst in b.instructions:
            si = getattr(inst, "sync_info", None)
            w = list(si.on_wait) if (si is not None and si.on_wait) else []
            if len(w) > 1:
                for extra in w[:-1]:
                    nop = make_nop(inst.engine)
                    nop.sync_info = bass_rust.SyncInfo(on_wait=[extra],
                                                       on_update=[])
                    out.append(nop)
                si.on_wait = [w[-1]]
                changed = True
            out.append(inst)
        if changed:
            b.instructions = out
    return nc


def _prep_common(x: np.ndarray):
    x = np.ascontiguousarray(x, dtype=np.float32)
    sq = (x.astype(np.float64) ** 2).sum(axis=1).astype(np.float32)   # (B,)
    xt = np.ascontiguousarray(x.T)                                    # (512, B)
    return x, sq, xt


def _prep_inputs_act(x: np.ndarray) -> list[dict[str, np.ndarray]]:
    x, sq, xt = _prep_common(x)
    in_maps = []
    for c in range(N_CORES):
        rot_b = np.roll(xt, -RPC * c, axis=1).astype(ml_dtypes.bfloat16)
        xt_c = np.ascontiguousarray(rot_b.reshape(K_TILES, 128, B))
        sq_rot = np.roll(sq, -RPC * c)
        sqh_c = np.ascontiguousarray(
            np.broadcast_to(-0.5 * sq_rot[None, :], (128, B))).astype(np.float32)
        sqhd_c = np.empty((128, M_TILES * 512), np.float32)
        for m in range(M_TILES):
            blk = sqh_c[:, (m // 4) * 512:(m // 4 + 1) * 512].copy()
            o = 128 * (m % 4)
            blk[np.arange(128), o + np.arange(128)] = -BIG
            sqhd_c[:, m * 512:(m + 1) * 512] = blk
        sqloc_c = np.ascontiguousarray(
            sq[RPC * c:RPC * (c + 1)].reshape(M_TILES, 128).T)        # (128, 8)
        in_maps.append({"xt": xt_c, "sqh": sqh_c, "sqhd": sqhd_c,
                        "sqloc": sqloc_c})
    return in_maps


def _prep_inputs_mm5(x: np.ndarray) -> list[dict[str, np.ndarray]]:
    x, sq, xt = _prep_common(x)
    sq_hi = sq.astype(ml_dtypes.bfloat16)
    sq_lo = (sq - sq_hi.astype(np.float32)).astype(ml_dtypes.bfloat16)
    ones2 = np.ones((2, 128), ml_dtypes.bfloat16)
    diagc = np.zeros((128, 128), np.float32)
    np.fill_diagonal(diagc, BIG)
    in_maps = []
    for c in range(N_CORES):
        rot_b = np.roll(xt, -RPC * c, axis=1).astype(ml_dtypes.bfloat16)
        xt_c = np.ascontiguousarray(
            rot_b.reshape(K_TILES, 128, N_TILES, 512).transpose(0, 2, 1, 3))
        xlt2_c = np.ascontiguousarray(rot_b[:, :RPC].reshape(K_TILES, 128, RPC)
                                      ) * np.float32(-2.0)
        sqr_c = np.stack([np.roll(sq_hi, -RPC * c), np.roll(sq_lo, -RPC * c)])
        sqloc_c = np.ascontiguousarray(
            sq[RPC * c:RPC * (c + 1)].reshape(M_TILES, 128).T)
        in_maps.append({"xt": xt_c, "xlt2": xlt2_c.astype(ml_dtypes.bfloat16),
                        "sqr": sqr_c, "ones2": ones2, "diagc": diagc,
                        "sqloc": sqloc_c})
    return in_maps


def _run(inputs: np.ndarray, trace: bool = False, variant: str | None = None):
    v = variant or VARIANT
    if v == "act_init":
        nc, in_maps = _build_act_init(), _prep_inputs_act(inputs)
    else:
        nc, in_maps = _build_mm5(), _prep_inputs_mm5(inputs)
    _split_waits(nc)
    res = run_bass_kernel_spmd(nc, in_maps, list(range(N_CORES)), trace=trace)
    lnd2 = np.stack([res.results[c]["lnd2"] for c in range(N_CORES)])
    loss = np.float32(-0.5 * np.mean(lnd2.astype(np.float64)))
    return np.asarray(loss, dtype=np.float32), res


def kernel(inputs: np.ndarray) -> np.ndarray:
    out, _ = _run(inputs, trace=False)
    return out
